# revision 9
# baseline (speedup 1.0000x reference)
"""MoE layer kernel for Trainium2 (8 NeuronCores, expert-parallel).

Reference computation (all fp32):
    gate:    h = relu(x@G1+g1); h = relu(h@G2+g2); logits = h@G3+g3   [N,E]
    topk:    top-2 softmax scattered to dense p [N,E]
    experts: hid = relu(x@W1[e]+b1[e]); out = hid@W2[e]+b2[e]         [E,N,O]
    combine: y = einsum('ne,eno->no', p, out)
    returns (y, p)

Sharding: 1 expert per core (E == n_cores == 8).  The gate runs
token-parallel (N/8 tokens per core) in fp32 so the top-2 selection
matches the fp32 reference, p is AllGathered on-device, then each core
runs its expert's MLP over ALL tokens in bf16 and scales by its own
column of p.  The host sums the 8 partial y's (the expert-axis
all-reduce) and concatenates the p chunks.
"""

from contextlib import ExitStack

import numpy as np
import ml_dtypes

import concourse.bass as bass
import concourse.bacc as bacc
import concourse.mybir as mybir
import concourse.tile as tile
from concourse import bass_utils

F32 = mybir.dt.float32
BF16 = mybir.dt.bfloat16
BF16_NP = ml_dtypes.bfloat16
AF = mybir.ActivationFunctionType
ALU = mybir.AluOpType


class Cfg:
    def __init__(self, D=1024, E=8, H=4096, O=1024, N=8192, NCORES=8,
                 TGC=512, TEC=512, expert_bf16=True):
        assert D % 128 == 0 and H % 128 == 0 and (4 * D) % 128 == 0
        assert O % 512 == 0 and N % 128 == 0
        assert E == NCORES
        self.D, self.E, self.H, self.O, self.N, self.NCORES = D, E, H, O, N, NCORES
        self.Tg = N // NCORES          # gate tokens per core
        self.TGC = min(TGC, self.Tg)   # gate token chunk
        self.TEC = TEC                 # expert token chunk
        assert self.Tg % self.TGC == 0 and self.TGC % 128 == 0
        assert N % TEC == 0 and TEC % 128 == 0
        self.F4 = 4 * D
        self.nD = D // 128
        self.nF4 = self.F4 // 128
        self.nH = H // 128
        self.nO2 = O // 512
        self.nCH = N // 128            # global 128-token chunks
        self.expert_bf16 = expert_bf16
        self.edt = BF16 if expert_bf16 else F32
        self.edt_np = BF16_NP if expert_bf16 else np.float32


def build_program(c: Cfg):
    """Emit the SPMD Bass/Tile program (identical on all cores; per-core
    behaviour differs only through input data)."""
    nc = bacc.Bacc("TRN2", target_bir_lowering=False, debug=False,
                   num_devices=c.NCORES)

    # ------------------------------ I/O ------------------------------
    xg = nc.dram_tensor("xg", [c.D, c.Tg], F32, kind="ExternalInput")
    g1m = nc.dram_tensor("g1m", [c.nF4, 128, c.nD, 128], F32, kind="ExternalInput")
    g2m = nc.dram_tensor("g2m", [c.nD, 128, c.nF4, 128], F32, kind="ExternalInput")
    g3m = nc.dram_tensor("g3m", [c.D, c.E], F32, kind="ExternalInput")
    g1s = nc.dram_tensor("g1s", [128, c.nF4], F32, kind="ExternalInput")
    g2s = nc.dram_tensor("g2s", [128, c.nD], F32, kind="ExternalInput")
    g3b = nc.dram_tensor("g3b", [128, c.E], F32, kind="ExternalInput")
    xt = nc.dram_tensor("xt", [c.D, c.N], c.edt, kind="ExternalInput")
    w1 = nc.dram_tensor("w1", [c.nH, 128, c.nD, 128], c.edt, kind="ExternalInput")
    w2 = nc.dram_tensor("w2", [c.H, c.O], c.edt, kind="ExternalInput")
    b1s = nc.dram_tensor("b1s", [128, c.nH], F32, kind="ExternalInput")
    b2b = nc.dram_tensor("b2b", [128, c.O], F32, kind="ExternalInput")
    oneh = nc.dram_tensor("oneh", [128, c.nCH * c.E], F32, kind="ExternalInput")
    y_part = nc.dram_tensor("y_part", [c.N, c.O], F32, kind="ExternalOutput")
    p_out = nc.dram_tensor("p_out", [c.Tg, c.E], F32, kind="ExternalOutput")

    NGC = c.Tg // c.TGC
    NEC = c.N // c.TEC

    with ExitStack() as ctx:
        tc = ctx.enter_context(tile.TileContext(nc))
        pers = ctx.enter_context(tc.tile_pool(name="pers", bufs=1))
        dram = ctx.enter_context(tc.tile_pool(name="dram", bufs=1, space="DRAM"))

        g1s_sb = pers.tile([128, c.nF4], F32)
        nc.sync.dma_start(g1s_sb[:], g1s[:])
        g2s_sb = pers.tile([128, c.nD], F32)
        nc.sync.dma_start(g2s_sb[:], g2s[:])
        g3b_sb = pers.tile([128, c.E], F32)
        nc.sync.dma_start(g3b_sb[:], g3b[:])
        g3_sb = pers.tile([128, c.nD, c.E], F32)
        nc.sync.dma_start(g3_sb[:], g3m[:].rearrange("(t p) e -> p t e", p=128))
        b1s_sb = pers.tile([128, c.nH], F32)
        nc.sync.dma_start(b1s_sb[:], b1s[:])
        b2b_sb = pers.tile([128, c.O], F32)
        nc.sync.dma_start(b2b_sb[:], b2b[:])
        oneh_sb = pers.tile([128, c.nCH, c.E], F32)
        nc.sync.dma_start(oneh_sb[:], oneh[:].rearrange("p (n e) -> p n e", e=c.E))

        p_int = dram.tile([c.Tg, c.E], F32)
        p_all = dram.tile([c.N, c.E], F32, addr_space="Shared")

        xg_r = xg[:].rearrange("(dt p) n -> p dt n", p=128)
        xt_r = xt[:].rearrange("(dt p) n -> p dt n", p=128)
        w2_r = w2[:].rearrange("(ht p) o -> p ht o", p=128)

        # ======================= GATE (fp32) =======================
        gp = ExitStack()
        xg_pool = gp.enter_context(tc.tile_pool(name="xg", bufs=2))
        g1_pool = gp.enter_context(tc.tile_pool(name="g1", bufs=3))
        g2_pool = gp.enter_context(tc.tile_pool(name="g2", bufs=2))
        h1_pool = gp.enter_context(tc.tile_pool(name="h1", bufs=1))
        h2_pool = gp.enter_context(tc.tile_pool(name="h2", bufs=1))
        tk_pool = gp.enter_context(tc.tile_pool(name="tk", bufs=2))
        ps1 = gp.enter_context(tc.tile_pool(name="ps1", bufs=2, space="PSUM"))
        ps3 = gp.enter_context(tc.tile_pool(name="ps3", bufs=2, space="PSUM"))

        gate_scope = nc.named_scope("gate")
        gate_scope.__enter__()
        for gc in range(NGC):
            tok0 = gc * c.TGC
            xgs = xg_pool.tile([128, c.nD, c.TGC], F32, tag="xgs")
            nc.sync.dma_start(xgs[:], xg_r[:, :, tok0:tok0 + c.TGC])

            # L1: h1 = relu(G1.T @ x + g1)   [F4, TGC]
            h1 = h1_pool.tile([128, c.nF4, c.TGC], F32, tag="h1")
            for ft in range(c.nF4):
                g1t = g1_pool.tile([128, c.nD, 128], F32, tag="g1t")
                nc.sync.dma_start(g1t[:], g1m[ft])
                acc = ps1.tile([128, c.TGC], F32, tag="ps1")
                for dt in range(c.nD):
                    nc.tensor.matmul(acc[:], g1t[:, dt, :], xgs[:, dt, :],
                                     start=(dt == 0), stop=(dt == c.nD - 1))
                nc.scalar.activation(h1[:, ft, :], acc[:], AF.Relu,
                                     bias=g1s_sb[:, ft:ft + 1])

            # L2: h2 = relu(G2.T @ h1 + g2)  [D, TGC]
            h2 = h2_pool.tile([128, c.nD, c.TGC], F32, tag="h2")
            for d2 in range(c.nD):
                g2t = g2_pool.tile([128, c.nF4, 128], F32, tag="g2t")
                nc.sync.dma_start(g2t[:], g2m[d2])
                acc = ps1.tile([128, c.TGC], F32, tag="ps1")
                for ft in range(c.nF4):
                    nc.tensor.matmul(acc[:], g2t[:, ft, :], h1[:, ft, :],
                                     start=(ft == 0), stop=(ft == c.nF4 - 1))
                nc.scalar.activation(h2[:, d2, :], acc[:], AF.Relu,
                                     bias=g2s_sb[:, d2:d2 + 1])

            # L3 + top-2 softmax, per 128-token subtile
            for t4 in range(c.TGC // 128):
                s0 = t4 * 128
                acc = ps3.tile([128, c.E], F32, tag="ps3")
                for d2 in range(c.nD):
                    nc.tensor.matmul(acc[:], h2[:, d2, s0:s0 + 128],
                                     g3_sb[:, d2, :],
                                     start=(d2 == 0), stop=(d2 == c.nD - 1))
                logits = tk_pool.tile([128, c.E], F32, tag="logits")
                nc.vector.tensor_tensor(logits[:], acc[:], g3b_sb[:], ALU.add)
                m1 = tk_pool.tile([128, 1], F32, tag="m1")
                nc.vector.reduce_max(m1[:], logits[:], mybir.AxisListType.X)
                mask1 = tk_pool.tile([128, c.E], F32, tag="mask1")
                nc.vector.tensor_scalar(mask1[:], logits[:], m1[:], None,
                                        ALU.is_ge)
                big1 = tk_pool.tile([128, c.E], F32, tag="big1")
                nc.vector.tensor_scalar(big1[:], mask1[:], 1e30, None, ALU.mult)
                masked = tk_pool.tile([128, c.E], F32, tag="masked")
                nc.vector.tensor_tensor(masked[:], logits[:], big1[:],
                                        ALU.subtract)
                m2 = tk_pool.tile([128, 1], F32, tag="m2")
                nc.vector.reduce_max(m2[:], masked[:], mybir.AxisListType.X)
                mask2 = tk_pool.tile([128, c.E], F32, tag="mask2")
                nc.vector.tensor_scalar(mask2[:], masked[:], m2[:], None,
                                        ALU.is_ge)
                # p1 = 1/(1+exp(m2-m1)); p2 = 1-p1
                dlt = tk_pool.tile([128, 1], F32, tag="dlt")
                nc.vector.tensor_tensor(dlt[:], m2[:], m1[:], ALU.subtract)
                es = tk_pool.tile([128, 1], F32, tag="es")
                nc.scalar.activation(es[:], dlt[:], AF.Exp)
                den = tk_pool.tile([128, 1], F32, tag="den")
                nc.vector.tensor_scalar(den[:], es[:], 1.0, None, ALU.add)
                p1 = tk_pool.tile([128, 1], F32, tag="p1")
                nc.vector.reciprocal(p1[:], den[:])
                p2 = tk_pool.tile([128, 1], F32, tag="p2")
                nc.vector.tensor_tensor(p2[:], es[:], p1[:], ALU.mult)
                pa = tk_pool.tile([128, c.E], F32, tag="pa")
                nc.vector.tensor_scalar(pa[:], mask1[:], p1[:], None, ALU.mult)
                pb = tk_pool.tile([128, c.E], F32, tag="pb")
                nc.vector.tensor_scalar(pb[:], mask2[:], p2[:], None, ALU.mult)
                pd = tk_pool.tile([128, c.E], F32, tag="pd")
                nc.vector.tensor_tensor(pd[:], pa[:], pb[:], ALU.add)
                row0 = tok0 + s0
                nc.sync.dma_start(p_int[row0:row0 + 128, :], pd[:])
                nc.sync.dma_start(p_out[row0:row0 + 128, :], pd[:])

        gp.close()
        gate_scope.__exit__(None, None, None)

        # ================== AllGather p + own column ==================
        ag_scope = nc.named_scope("agp")
        ag_scope.__enter__()
        nc.gpsimd.collective_compute(
            "AllGather",
            ALU.bypass,
            replica_groups=[list(range(c.NCORES))],
            ins=[p_int[:].opt()],
            outs=[p_all[:].opt()],
        )
        p_sb = pers.tile([128, c.nCH, c.E], F32)
        nc.sync.dma_start(p_sb[:], p_all[:].rearrange("(n p) e -> p n e", p=128))
        prod = pers.tile([128, c.nCH, c.E], F32)
        nc.vector.tensor_tensor(prod[:], p_sb[:], oneh_sb[:], ALU.mult)
        pcol = pers.tile([128, c.nCH, 1], F32)
        nc.vector.tensor_reduce(pcol[:], prod[:], mybir.AxisListType.X, ALU.add)
        ag_scope.__exit__(None, None, None)

        # ======================= EXPERT (bf16) =======================
        exp_scope = nc.named_scope("expert")
        exp_scope.__enter__()
        ep = ExitStack()
        xt_pool = ep.enter_context(tc.tile_pool(name="xt", bufs=2))
        w1_pool = ep.enter_context(tc.tile_pool(name="w1", bufs=3))
        w2_pool = ep.enter_context(tc.tile_pool(name="w2", bufs=2))
        hid_pool = ep.enter_context(tc.tile_pool(name="hid", bufs=1))
        y_pool = ep.enter_context(tc.tile_pool(name="y", bufs=3))
        psA = ep.enter_context(tc.tile_pool(name="psA", bufs=2, space="PSUM"))
        psB = ep.enter_context(tc.tile_pool(name="psB", bufs=2, space="PSUM"))

        for ec in range(NEC):
            tok0 = ec * c.TEC
            xts = xt_pool.tile([128, c.nD, c.TEC], c.edt, tag="xts")
            nc.sync.dma_start(xts[:], xt_r[:, :, tok0:tok0 + c.TEC])

            # Phase A: hid = relu(W1.T @ x + b1)   [H, TEC]
            hid = hid_pool.tile([128, c.nH, c.TEC], c.edt, tag="hid")
            for ht in range(c.nH):
                w1t = w1_pool.tile([128, c.nD, 128], c.edt, tag="w1t")
                nc.sync.dma_start(w1t[:], w1[ht])
                acc = psA.tile([128, c.TEC], F32, tag="psA")
                for dt in range(c.nD):
                    nc.tensor.matmul(acc[:], w1t[:, dt, :], xts[:, dt, :],
                                     start=(dt == 0), stop=(dt == c.nD - 1))
                nc.scalar.activation(hid[:, ht, :], acc[:], AF.Relu,
                                     bias=b1s_sb[:, ht:ht + 1])

            # Phase B: y_part = p_c * (hid.T @ W2 + b2)   [TEC, O]
            for ot in range(c.nO2):
                o0 = ot * 512
                w2t = w2_pool.tile([128, c.nH, 512], c.edt, tag="w2t")
                nc.sync.dma_start(w2t[:], w2_r[:, :, o0:o0 + 512])
                for t4 in range(c.TEC // 128):
                    s0 = t4 * 128
                    acc = psB.tile([128, 512], F32, tag="psB")
                    for ht in range(c.nH):
                        nc.tensor.matmul(acc[:], hid[:, ht, s0:s0 + 128],
                                         w2t[:, ht, :],
                                         start=(ht == 0), stop=(ht == c.nH - 1))
                    yt = y_pool.tile([128, 512], F32, tag="yt")
                    nc.vector.tensor_tensor(yt[:], acc[:],
                                            b2b_sb[:, o0:o0 + 512], ALU.add)
                    g = (tok0 + s0) // 128
                    yt2 = y_pool.tile([128, 512], F32, tag="yt2")
                    nc.scalar.activation(yt2[:], yt[:], AF.Copy,
                                         scale=pcol[:, g, :])
                    row0 = tok0 + s0
                    nc.sync.dma_start(y_part[row0:row0 + 128, o0:o0 + 512],
                                      yt2[:])
        ep.close()
        exp_scope.__exit__(None, None, None)

    nc.compile()
    return nc


# --------------------------- host wrapper ---------------------------

def prepare_inputs(c, x, W1, b1, W2, b2, G1, g1, G2, g2, G3, g3):
    """Build the per-core input maps (all host-side numpy prep)."""
    x = np.asarray(x, np.float32)
    xT = np.ascontiguousarray(x.T)                       # [D, N]
    xt_e = np.ascontiguousarray(xT.astype(c.edt_np))

    G1 = np.asarray(G1, np.float32)
    G2 = np.asarray(G2, np.float32)
    g1p = np.ascontiguousarray(
        G1.reshape(c.nD, 128, c.nF4, 128).transpose(2, 1, 0, 3))
    g2p = np.ascontiguousarray(
        G2.reshape(c.nF4, 128, c.nD, 128).transpose(2, 1, 0, 3))
    g3m = np.ascontiguousarray(np.asarray(G3, np.float32))
    g1s = np.ascontiguousarray(np.asarray(g1, np.float32).reshape(c.nF4, 128).T)
    g2s = np.ascontiguousarray(np.asarray(g2, np.float32).reshape(c.nD, 128).T)
    g3b = np.ascontiguousarray(
        np.broadcast_to(np.asarray(g3, np.float32), (128, c.E)))

    W1 = np.asarray(W1, np.float32)
    W2 = np.asarray(W2, np.float32)
    b1 = np.asarray(b1, np.float32)
    b2 = np.asarray(b2, np.float32)

    in_maps = []
    for core in range(c.NCORES):
        w1p = np.ascontiguousarray(
            W1[core].astype(c.edt_np)
            .reshape(c.nD, 128, c.nH, 128).transpose(2, 1, 0, 3))
        w2p = np.ascontiguousarray(W2[core].astype(c.edt_np))
        b1sc = np.ascontiguousarray(b1[core].reshape(c.nH, 128).T)
        b2bc = np.ascontiguousarray(np.broadcast_to(b2[core], (128, c.O)))
        onehot = (np.arange(c.E) == core).astype(np.float32)
        oneh = np.ascontiguousarray(
            np.broadcast_to(np.tile(onehot, c.nCH), (128, c.nCH * c.E)))
        xgc = np.ascontiguousarray(xT[:, core * c.Tg:(core + 1) * c.Tg])
        in_maps.append({
            "xg": xgc, "g1m": g1p, "g2m": g2p, "g3m": g3m,
            "g1s": g1s, "g2s": g2s, "g3b": g3b,
            "xt": xt_e, "w1": w1p, "w2": w2p,
            "b1s": b1sc, "b2b": b2bc, "oneh": oneh,
        })
    return in_maps


_PROGRAM_CACHE = {}


def run_moe(c, inputs, trace=False):
    key = (c.D, c.E, c.H, c.O, c.N, c.NCORES, c.TGC, c.TEC, c.expert_bf16)
    if key not in _PROGRAM_CACHE:
        _PROGRAM_CACHE[key] = build_program(c)
    nc = _PROGRAM_CACHE[key]
    in_maps = prepare_inputs(c, **inputs)
    res = bass_utils.run_bass_kernel_spmd(
        nc, in_maps, core_ids=list(range(c.NCORES)), trace=trace)
    acc = np.zeros((c.N, c.O), np.float64)
    for r in res.results:
        acc += r["y_part"].astype(np.float64)
    y = acc.astype(np.float32)
    p = np.concatenate([r["p_out"] for r in res.results], axis=0)
    return (y, p), res


def kernel(**inputs):
    cfg = Cfg()
    (y, p), _ = run_moe(cfg, inputs, trace=False)
    return (y, p)


# revision 10
# speedup vs baseline: 2.2079x; 2.2079x over previous
"""MoE layer kernel for Trainium2 (8 NeuronCores).

Two-launch design:
  Launch G (gate): token-parallel fp32 3-layer MLP -> logits [N, E].
  Host: top-2 softmax -> dense p; build balanced expert-pure blocks of
        512 (token, expert) pairs; 5 blocks per core (worst-case bound
        sum_e ceil(n_e/512) <= N*2/512 + E = 40 blocks for this size).
  Launch E (experts): each core runs 5 segments; segment s computes
        relu(x_gathered @ W1[e_s] + b1[e_s]) @ W2[e_s] scaled by the
        pair's gate weight, in bf16.
  Host: scatter-add segment outputs into y, plus the p @ b2 term
        (sum_e p[n,e]*b2[e], exact because sum of selected p = weights).

This exploits top-2 routing sparsity (4x fewer expert FLOPs than the
dense reference evaluation) while staying load-balanced under arbitrary
routing skew, and returns outputs identical to the dense formula since
terms with p[n,e] == 0 contribute nothing.
"""

from contextlib import ExitStack

import numpy as np
import ml_dtypes

import concourse.bass as bass
import concourse.bacc as bacc
import concourse.mybir as mybir
import concourse.tile as tile
from concourse import bass_utils

F32 = mybir.dt.float32
BF16 = mybir.dt.bfloat16
BF16_NP = ml_dtypes.bfloat16
AF = mybir.ActivationFunctionType
ALU = mybir.AluOpType


class Cfg:
    def __init__(self, D=1024, E=8, H=4096, O=1024, N=8192, NCORES=8,
                 TGC=512, SEG=512):
        assert D % 128 == 0 and H % 128 == 0 and (4 * D) % 128 == 0
        assert O % 512 == 0 and N % 128 == 0
        self.D, self.E, self.H, self.O, self.N, self.NCORES = D, E, H, O, N, NCORES
        self.Tg = N // NCORES
        self.TGC = min(TGC, self.Tg)
        assert self.Tg % self.TGC == 0 and self.TGC % 128 == 0
        self.F4 = 4 * D
        self.nD = D // 128
        self.nF4 = self.F4 // 128
        self.nH = H // 128
        self.nO2 = O // 512
        self.SEG = SEG                     # pairs per block/segment
        nblocks = (2 * N) // SEG + E       # worst-case expert-pure blocks
        self.NSEG = -(-nblocks // NCORES)  # segments per core
        self.CAP = self.NSEG * SEG         # padded pairs per core


# ======================= gate program (fp32) =======================

def build_gate_program(c: Cfg):
    nc = bacc.Bacc("TRN2", target_bir_lowering=False, debug=False,
                   num_devices=c.NCORES)
    xg = nc.dram_tensor("xg", [c.D, c.Tg], F32, kind="ExternalInput")
    g1m = nc.dram_tensor("g1m", [c.nF4, 128, c.nD, 128], F32, kind="ExternalInput")
    g2m = nc.dram_tensor("g2m", [c.nD, 128, c.nF4, 128], F32, kind="ExternalInput")
    g3m = nc.dram_tensor("g3m", [c.D, c.E], F32, kind="ExternalInput")
    g1s = nc.dram_tensor("g1s", [128, c.nF4], F32, kind="ExternalInput")
    g2s = nc.dram_tensor("g2s", [128, c.nD], F32, kind="ExternalInput")
    g3b = nc.dram_tensor("g3b", [128, c.E], F32, kind="ExternalInput")
    lg_out = nc.dram_tensor("lg_out", [c.Tg, c.E], F32, kind="ExternalOutput")

    NGC = c.Tg // c.TGC
    with ExitStack() as ctx:
        tc = ctx.enter_context(tile.TileContext(nc))
        pers = ctx.enter_context(tc.tile_pool(name="pers", bufs=1))
        xg_pool = ctx.enter_context(tc.tile_pool(name="xg", bufs=2))
        g1_pool = ctx.enter_context(tc.tile_pool(name="g1", bufs=3))
        g2_pool = ctx.enter_context(tc.tile_pool(name="g2", bufs=2))
        h1_pool = ctx.enter_context(tc.tile_pool(name="h1", bufs=1))
        h2_pool = ctx.enter_context(tc.tile_pool(name="h2", bufs=1))
        lg_pool = ctx.enter_context(tc.tile_pool(name="lg", bufs=2))
        ps1 = ctx.enter_context(tc.tile_pool(name="ps1", bufs=2, space="PSUM"))
        ps3 = ctx.enter_context(tc.tile_pool(name="ps3", bufs=2, space="PSUM"))

        g1s_sb = pers.tile([128, c.nF4], F32)
        nc.sync.dma_start(g1s_sb[:], g1s[:])
        g2s_sb = pers.tile([128, c.nD], F32)
        nc.sync.dma_start(g2s_sb[:], g2s[:])
        g3b_sb = pers.tile([128, c.E], F32)
        nc.sync.dma_start(g3b_sb[:], g3b[:])
        g3_sb = pers.tile([128, c.nD, c.E], F32)
        nc.sync.dma_start(g3_sb[:], g3m[:].rearrange("(t p) e -> p t e", p=128))

        xg_r = xg[:].rearrange("(dt p) n -> p dt n", p=128)

        for gc in range(NGC):
            tok0 = gc * c.TGC
            xgs = xg_pool.tile([128, c.nD, c.TGC], F32, tag="xgs")
            nc.sync.dma_start(xgs[:], xg_r[:, :, tok0:tok0 + c.TGC])

            h1 = h1_pool.tile([128, c.nF4, c.TGC], F32, tag="h1")
            for ft in range(c.nF4):
                g1t = g1_pool.tile([128, c.nD, 128], F32, tag="g1t")
                nc.sync.dma_start(g1t[:], g1m[ft])
                acc = ps1.tile([128, c.TGC], F32, tag="ps1")
                for dt in range(c.nD):
                    nc.tensor.matmul(acc[:], g1t[:, dt, :], xgs[:, dt, :],
                                     start=(dt == 0), stop=(dt == c.nD - 1))
                nc.scalar.activation(h1[:, ft, :], acc[:], AF.Relu,
                                     bias=g1s_sb[:, ft:ft + 1])

            h2 = h2_pool.tile([128, c.nD, c.TGC], F32, tag="h2")
            for d2 in range(c.nD):
                g2t = g2_pool.tile([128, c.nF4, 128], F32, tag="g2t")
                nc.sync.dma_start(g2t[:], g2m[d2])
                acc = ps1.tile([128, c.TGC], F32, tag="ps1")
                for ft in range(c.nF4):
                    nc.tensor.matmul(acc[:], g2t[:, ft, :], h1[:, ft, :],
                                     start=(ft == 0), stop=(ft == c.nF4 - 1))
                nc.scalar.activation(h2[:, d2, :], acc[:], AF.Relu,
                                     bias=g2s_sb[:, d2:d2 + 1])

            for t4 in range(c.TGC // 128):
                s0 = t4 * 128
                acc = ps3.tile([128, c.E], F32, tag="ps3")
                for d2 in range(c.nD):
                    nc.tensor.matmul(acc[:], h2[:, d2, s0:s0 + 128],
                                     g3_sb[:, d2, :],
                                     start=(d2 == 0), stop=(d2 == c.nD - 1))
                logits = lg_pool.tile([128, c.E], F32, tag="logits")
                nc.vector.tensor_tensor(logits[:], acc[:], g3b_sb[:], ALU.add)
                row0 = tok0 + s0
                nc.sync.dma_start(lg_out[row0:row0 + 128, :], logits[:])

    nc.compile()
    return nc


# ==================== sparse expert program (bf16) ====================

def build_expert_program(c: Cfg):
    nc = bacc.Bacc("TRN2", target_bir_lowering=False, debug=False,
                   num_devices=c.NCORES)
    S = c.SEG
    xe = nc.dram_tensor("xe", [c.D, c.CAP], BF16, kind="ExternalInput")
    w1 = nc.dram_tensor("w1", [c.NSEG, c.nH, 128, c.nD, 128], BF16,
                        kind="ExternalInput")
    w2 = nc.dram_tensor("w2", [c.NSEG, c.H, c.O], BF16, kind="ExternalInput")
    b1s = nc.dram_tensor("b1s", [c.NSEG, 128, c.nH], F32, kind="ExternalInput")
    pw = nc.dram_tensor("pw", [128, c.CAP // 128], F32, kind="ExternalInput")
    ye = nc.dram_tensor("ye", [c.CAP, c.O], F32, kind="ExternalOutput")

    with ExitStack() as ctx:
        tc = ctx.enter_context(tile.TileContext(nc))
        pers = ctx.enter_context(tc.tile_pool(name="pers", bufs=1))
        xe_pool = ctx.enter_context(tc.tile_pool(name="xe", bufs=2))
        w1_pool = ctx.enter_context(tc.tile_pool(name="w1", bufs=3))
        w2_pool = ctx.enter_context(tc.tile_pool(name="w2", bufs=2))
        b1_pool = ctx.enter_context(tc.tile_pool(name="b1", bufs=2))
        hid_pool = ctx.enter_context(tc.tile_pool(name="hid", bufs=1))
        y_pool = ctx.enter_context(tc.tile_pool(name="y", bufs=4))
        psA = ctx.enter_context(tc.tile_pool(name="psA", bufs=2, space="PSUM"))
        psB = ctx.enter_context(tc.tile_pool(name="psB", bufs=2, space="PSUM"))

        pw_sb = pers.tile([128, c.CAP // 128], F32)
        nc.sync.dma_start(pw_sb[:], pw[:])

        xe_r = xe[:].rearrange("(dt p) n -> p dt n", p=128)

        for sg in range(c.NSEG):
            tok0 = sg * S
            xts = xe_pool.tile([128, c.nD, S], BF16, tag="xts")
            nc.sync.dma_start(xts[:], xe_r[:, :, tok0:tok0 + S])
            b1t = b1_pool.tile([128, c.nH], F32, tag="b1t")
            nc.sync.dma_start(b1t[:], b1s[sg])

            # Phase A: hid = relu(W1.T @ x + b1)   [H, S]
            hid = hid_pool.tile([128, c.nH, S], BF16, tag="hid")
            for ht in range(c.nH):
                w1t = w1_pool.tile([128, c.nD, 128], BF16, tag="w1t")
                nc.sync.dma_start(w1t[:], w1[sg, ht])
                acc = psA.tile([128, S], F32, tag="psA")
                for dt in range(c.nD):
                    nc.tensor.matmul(acc[:], w1t[:, dt, :], xts[:, dt, :],
                                     start=(dt == 0), stop=(dt == c.nD - 1))
                nc.scalar.activation(hid[:, ht, :], acc[:], AF.Relu,
                                     bias=b1t[:, ht:ht + 1])

            # Phase B: ye = pw * (hid.T @ W2)   [S, O]
            w2_r = w2[sg].rearrange("(ht p) o -> p ht o", p=128)
            for ot in range(c.nO2):
                o0 = ot * 512
                w2t = w2_pool.tile([128, c.nH, 512], BF16, tag="w2t")
                nc.sync.dma_start(w2t[:], w2_r[:, :, o0:o0 + 512])
                for t4 in range(S // 128):
                    s0 = t4 * 128
                    acc = psB.tile([128, 512], F32, tag="psB")
                    for ht in range(c.nH):
                        nc.tensor.matmul(acc[:], hid[:, ht, s0:s0 + 128],
                                         w2t[:, ht, :],
                                         start=(ht == 0), stop=(ht == c.nH - 1))
                    yt = y_pool.tile([128, 512], F32, tag="yt")
                    g = (tok0 + s0) // 128
                    nc.vector.tensor_scalar(yt[:], acc[:],
                                            pw_sb[:, g:g + 1], None, ALU.mult)
                    row0 = tok0 + s0
                    nc.sync.dma_start(ye[row0:row0 + 128, o0:o0 + 512], yt[:])

    nc.compile()
    return nc


# --------------------------- host side ---------------------------

def host_route(c, logits, b2):
    """Top-2 softmax -> dense p; balanced expert-pure blocks."""
    N, E = logits.shape
    idx = np.argsort(-logits, axis=1, kind="stable")[:, :2]
    topv = np.take_along_axis(logits, idx, axis=1)
    ex = np.exp(topv - topv[:, :1])
    tp = (ex / ex.sum(1, keepdims=True)).astype(np.float32)
    p = np.zeros_like(logits)
    np.put_along_axis(p, idx, tp, axis=1)

    # (token, expert, weight) pairs grouped by expert, padded to blocks of SEG
    S = c.SEG
    blocks = []          # (expert, token_idx[S], weight[S], valid_count)
    for e in range(E):
        tok = np.concatenate([np.nonzero(idx[:, k] == e)[0] for k in range(2)])
        wgt = p[tok, e]
        n = len(tok)
        npad = -(-max(n, 1) // S) * S
        tok_p = np.zeros(npad, np.int64)
        tok_p[:n] = tok
        wgt_p = np.zeros(npad, np.float32)
        wgt_p[:n] = wgt
        for b in range(npad // S):
            blocks.append((e, tok_p[b * S:(b + 1) * S], wgt_p[b * S:(b + 1) * S]))
    while len(blocks) < c.NSEG * c.NCORES:
        blocks.append((0, np.zeros(S, np.int64), np.zeros(S, np.float32)))
    assert len(blocks) <= c.NSEG * c.NCORES, \
        f"block overflow: {len(blocks)} > {c.NSEG * c.NCORES}"
    return p, blocks


def prepare_gate_inputs(c, xT, G1, g1, G2, g2, G3, g3):
    g1p = np.ascontiguousarray(
        G1.reshape(c.nD, 128, c.nF4, 128).transpose(2, 1, 0, 3))
    g2p = np.ascontiguousarray(
        G2.reshape(c.nF4, 128, c.nD, 128).transpose(2, 1, 0, 3))
    g1s = np.ascontiguousarray(g1.reshape(c.nF4, 128).T)
    g2s = np.ascontiguousarray(g2.reshape(c.nD, 128).T)
    g3b = np.ascontiguousarray(np.broadcast_to(g3, (128, c.E)))
    maps = []
    for core in range(c.NCORES):
        xgc = np.ascontiguousarray(xT[:, core * c.Tg:(core + 1) * c.Tg])
        maps.append({"xg": xgc, "g1m": g1p, "g2m": g2p, "g3m": G3,
                     "g1s": g1s, "g2s": g2s, "g3b": g3b})
    return maps


def prepare_expert_inputs(c, xT_bf, W1bf, W2bf, b1, blocks):
    """W1bf: [E, nH, 128, nD, 128] packed bf16; W2bf: [E, H, O] bf16."""
    b1r = np.ascontiguousarray(
        b1.reshape(c.E, c.nH, 128).transpose(0, 2, 1))       # [E, 128, nH]
    maps = []
    for core in range(c.NCORES):
        bl = blocks[core * c.NSEG:(core + 1) * c.NSEG]
        toks = np.concatenate([b[1] for b in bl])            # [CAP]
        wgts = np.concatenate([b[2] for b in bl])            # [CAP]
        xe = np.ascontiguousarray(xT_bf[:, toks])            # [D, CAP]
        w1c = np.ascontiguousarray(
            np.stack([W1bf[b[0]] for b in bl]))              # [NSEG,...]
        w2c = np.ascontiguousarray(np.stack([W2bf[b[0]] for b in bl]))
        b1c = np.ascontiguousarray(np.stack([b1r[b[0]] for b in bl]))
        pwc = np.ascontiguousarray(
            wgts.reshape(c.CAP // 128, 128).T)               # [128, CAP/128]
        maps.append({"xe": xe, "w1": w1c, "w2": w2c, "b1s": b1c, "pw": pwc})
    return maps


_CACHE = {}


def _programs(c):
    key = (c.D, c.E, c.H, c.O, c.N, c.NCORES, c.TGC, c.SEG)
    if key not in _CACHE:
        _CACHE[key] = (build_gate_program(c), build_expert_program(c))
    return _CACHE[key]


def run_moe(c, inputs, trace=False):
    x = np.asarray(inputs["x"], np.float32)
    W1 = np.asarray(inputs["W1"], np.float32)
    b1 = np.asarray(inputs["b1"], np.float32)
    W2 = np.asarray(inputs["W2"], np.float32)
    b2 = np.asarray(inputs["b2"], np.float32)
    G1 = np.asarray(inputs["G1"], np.float32)
    g1 = np.asarray(inputs["g1"], np.float32)
    G2 = np.asarray(inputs["G2"], np.float32)
    g2 = np.asarray(inputs["g2"], np.float32)
    G3 = np.asarray(inputs["G3"], np.float32)
    g3 = np.asarray(inputs["g3"], np.float32)

    gate_nc, exp_nc = _programs(c)

    xT = np.ascontiguousarray(x.T)
    gmaps = prepare_gate_inputs(c, xT, G1, g1, G2, g2, G3, g3)
    gres = bass_utils.run_bass_kernel_spmd(
        gate_nc, gmaps, core_ids=list(range(c.NCORES)), trace=trace)
    logits = np.concatenate([r["lg_out"] for r in gres.results], axis=0)

    p, blocks = host_route(c, logits, b2)

    xT_bf = xT.astype(BF16_NP)
    W1bf = np.stack([np.ascontiguousarray(
        W1[e].astype(BF16_NP).reshape(c.nD, 128, c.nH, 128)
        .transpose(2, 1, 0, 3)) for e in range(c.E)])
    W2bf = W2.astype(BF16_NP)
    emaps = prepare_expert_inputs(c, xT_bf, W1bf, W2bf, b1, blocks)
    eres = bass_utils.run_bass_kernel_spmd(
        exp_nc, emaps, core_ids=list(range(c.NCORES)), trace=trace)

    y = p @ b2
    for core in range(c.NCORES):
        ye = eres.results[core]["ye"]
        bl = blocks[core * c.NSEG:(core + 1) * c.NSEG]
        for s, (e, toks, wgts) in enumerate(bl):
            valid = wgts != 0
            rows = ye[s * c.SEG:(s + 1) * c.SEG]
            y[toks[valid]] += rows[valid]
    return (y, p), (gres, eres)


def kernel(**inputs):
    cfg = Cfg()
    (y, p), _ = run_moe(cfg, inputs, trace=False)
    return (y, p)
